# revision 18
# baseline (speedup 1.0000x reference)
import sys
sys.path.insert(0, '/opt/trn_rl_repo')
import numpy as np
import ml_dtypes

BF = ml_dtypes.bfloat16
F32 = np.float32
N_CORES = 8
S_IMG, S_TXT, S_BOK = 2048, 512, 4
S = S_TXT + S_IMG  # 2560, order [enc, img]
DM, H, DH, R = 3072, 24, 128, 16
NH = H // N_CORES  # 3 heads per core
DCH = NH * DH      # 384 cols per core
EPS = 1e-6
SCALE = 1.0 / np.sqrt(DH)
NKT = DM // 128    # 24
NQC = S // 512     # 5
NST = S // 128     # 20


def _swap_pairs(w):
    v = w.reshape(-1, 2)
    return np.stack([v[:, 1], v[:, 0]], axis=1).reshape(-1)


def _prep(inputs):
    """Host-side sharding/layout prep. Returns list of per-core input dicts."""
    hs = np.asarray(inputs['hidden_states'])[0]      # [2048, 3072]
    enc = np.asarray(inputs['encoder_hidden_states'])[0]  # [512, 3072]
    bok = np.asarray(inputs['bokeh_embeds'])[0]      # [4, 3072]
    cos = np.asarray(inputs['rope_cos'])             # [2560, 128]
    sin = np.asarray(inputs['rope_sin'])
    g = lambda n: np.asarray(inputs[n])

    xcat = np.concatenate([enc, hs], axis=0)         # [2560, 3072]
    xt = np.ascontiguousarray(xcat.T).astype(BF)     # [3072, 2560]

    # LoRA weight fusion: down(x@W.T+b) = x @ (down@W).T + down@b
    Aq = (g('qadp_down') @ g('Wq'))                  # [16, 3072]
    Ak = (g('kadp_down') @ g('Wk'))
    cq = (g('qadp_down') @ g('bq')).astype(F32).reshape(16, 1)
    ck = (g('kadp_down') @ g('bk')).astype(F32).reshape(16, 1)

    cosT = np.ascontiguousarray(cos.T).astype(F32)   # [128, 2560]
    sinT = np.ascontiguousarray(sin.T).astype(F32)

    def rope_consts(w_enc, w_img):
        cw = np.empty((128, S), F32)
        sw = np.empty((128, S), F32)
        cw[:, :S_TXT] = cosT[:, :S_TXT] * w_enc[:, None]
        cw[:, S_TXT:] = cosT[:, S_TXT:] * w_img[:, None]
        sw[:, :S_TXT] = sinT[:, :S_TXT] * _swap_pairs(w_enc)[:, None]
        sw[:, S_TXT:] = sinT[:, S_TXT:] * _swap_pairs(w_img)[:, None]
        return cw, sw

    cosw_q, sinw_q = rope_consts(g('norm_aq_w').astype(F32), g('norm_q_w').astype(F32))
    cosw_k, sinw_k = rope_consts(g('norm_ak_w').astype(F32), g('norm_k_w').astype(F32))

    # permutation for rope rotation: rot = Perm @ qn (rot[2i]=-qn[2i+1], rot[2i+1]=qn[2i])
    perm = np.zeros((128, 128), F32)
    for i in range(64):
        perm[2 * i, 2 * i + 1] = -1.0
        perm[2 * i + 1, 2 * i] = 1.0
    permT = np.ascontiguousarray(perm.T)             # lhsT for matmul

    bokT = np.ascontiguousarray(bok.T).astype(BF)    # [3072, 4]
    dbk = np.ascontiguousarray(g('kb_down').T).astype(BF)   # [3072, 16]
    dbv = np.ascontiguousarray(g('vb_down').T).astype(BF)

    per_core = []
    for c in range(N_CORES):
        c0 = c * DCH
        hsl = slice(c0, c0 + DCH)
        d = {}
        d['xt'] = xt
        d['wq'] = np.ascontiguousarray(g('Wq')[hsl, :].T).astype(BF)   # [3072, 384]
        d['wk'] = np.ascontiguousarray(g('Wk')[hsl, :].T).astype(BF)
        d['waq'] = np.ascontiguousarray(g('Waq')[hsl, :].T).astype(BF)
        d['wak'] = np.ascontiguousarray(g('Wak')[hsl, :].T).astype(BF)
        d['wv'] = np.ascontiguousarray(g('Wv')[hsl, :].T).astype(BF)
        d['wav'] = np.ascontiguousarray(g('Wav')[hsl, :].T).astype(BF)
        d['aq'] = np.ascontiguousarray(Aq.T).astype(BF)                # [3072, 16]
        d['ak'] = np.ascontiguousarray(Ak.T).astype(BF)
        d['uq'] = np.ascontiguousarray(g('qadp_up')[hsl, :].T).astype(BF)  # [16, 384]
        d['uk'] = np.ascontiguousarray(g('kadp_up')[hsl, :].T).astype(BF)
        d['cq'] = cq
        d['ck'] = ck
        d['bq_t'] = np.ascontiguousarray(g('bq')[hsl].reshape(NH, 128).T).astype(F32)  # [128,3]
        d['bk_t'] = np.ascontiguousarray(g('bk')[hsl].reshape(NH, 128).T).astype(F32)
        d['baq_t'] = np.ascontiguousarray(g('baq')[hsl].reshape(NH, 128).T).astype(F32)
        d['bak_t'] = np.ascontiguousarray(g('bak')[hsl].reshape(NH, 128).T).astype(F32)
        d['bv_row'] = g('bv')[hsl].reshape(1, DCH).astype(BF)
        d['bav_row'] = g('bav')[hsl].reshape(1, DCH).astype(BF)
        d['cosw_q'] = cosw_q
        d['sinw_q'] = sinw_q
        d['cosw_k'] = cosw_k
        d['sinw_k'] = sinw_k
        d['perm'] = permT.astype(BF)
        d['bok_t'] = bokT
        d['dbk'] = dbk
        d['dbv'] = dbv
        d['ukb'] = np.ascontiguousarray(g('kb_up')[hsl, :].T).astype(BF)   # [16, 384]
        d['uvb'] = np.ascontiguousarray(g('vb_up')[hsl, :].T).astype(BF)
        d['wo_t'] = np.ascontiguousarray(g('Wo')[:, hsl].T).astype(BF)     # [384, 3072]
        d['wao_t'] = np.ascontiguousarray(g('Wao')[:, hsl].T).astype(BF)
        per_core.append(d)
    return per_core


def _mock_core(d):
    """Numpy emulation of the device program for one core (bf16 where device is bf16)."""
    f = lambda a: np.asarray(a, F32)
    xt = f(d['xt'])                       # [3072, 2560]
    qt_heads, kt_heads = [], []
    downq = f(d['aq']).T @ xt + d['cq']   # [16, 2560]
    downk = f(d['ak']).T @ xt + d['ck']
    for h in range(NH):
        hs = slice(h * 128, (h + 1) * 128)
        for (wi, we, u, dwn, bt, bet, cw, sw, out) in (
            ('wq', 'waq', 'uq', downq, 'bq_t', 'baq_t', 'cosw_q', 'sinw_q', qt_heads),
            ('wk', 'wak', 'uk', downk, 'bk_t', 'bak_t', 'cosw_k', 'sinw_k', kt_heads)):
            qt = np.empty((128, S), F32)
            qt[:, :S_TXT] = f(d[we])[:, hs].T @ xt[:, :S_TXT] + d[bet][:, h:h + 1]
            qt[:, S_TXT:] = (f(d[wi])[:, hs].T @ xt[:, S_TXT:]
                             + np.asarray(f(d[u])[:, hs].T @ dwn[:, S_TXT:].astype(BF), F32)
                             + d[bt][:, h:h + 1])
            ss = (qt * qt).sum(axis=0, keepdims=True)          # [1, S]
            r = 1.0 / np.sqrt(ss / 128.0 + EPS)
            qn = qt * r
            rot = f(d['perm']).T @ qn
            qr = qn * d[cw] + rot * d[sw]
            out.append(np.asarray(qr, BF).astype(F32))
    # v
    v = np.empty((S, DCH), F32)
    v[:S_TXT] = xt[:, :S_TXT].T @ f(d['wav']) + f(d['bav_row'])
    v[S_TXT:] = xt[:, S_TXT:].T @ f(d['wv']) + f(d['bv_row'])
    v = np.asarray(v, BF).astype(F32)
    # bokeh
    downbk = f(d['dbk']).T @ f(d['bok_t'])     # [16, 4]
    downbv = f(d['dbv']).T @ f(d['bok_t'])
    kdT = np.asarray(f(d['ukb']).T @ np.asarray(downbk, BF).astype(F32), BF).astype(F32)  # [384, 4]
    vd = np.asarray(np.asarray(downbv, BF).astype(F32).T @ f(d['uvb']), BF).astype(F32)   # [4, 384]
    att = np.empty((128, S, NH), F32)
    for h in range(NH):
        hs = slice(h * 128, (h + 1) * 128)
        qtr, ktr = qt_heads[h], kt_heads[h]
        sc = ktr.T @ qtr                                   # [k=S, q=S]
        p = np.exp(sc * SCALE, dtype=F32)
        p_bf = np.asarray(p, BF).astype(F32)
        s_m = p_bf.sum(axis=0, keepdims=True)              # [1, S]
        attu = v[:, hs].T @ p_bf                           # [128, S]
        scb = kdT[hs, :].T @ qtr                           # [4, S]
        pb = np.exp(scb * SCALE, dtype=F32)
        pb_bf = np.asarray(pb, BF).astype(F32)
        s_b = pb_bf.sum(axis=0, keepdims=True)
        camu = vd[:, hs].T @ pb_bf                         # [128, S]
        tot = attu * (1.0 / s_m) + camu * (1.0 / s_b)
        att[:, :, h] = np.asarray(tot, BF).astype(F32)
    # out-proj
    po = np.empty((S, DM), F32)
    w2e = f(d['wao_t'])  # [384, 3072]
    w2i = f(d['wo_t'])
    attf = att.transpose(2, 0, 1).reshape(DCH, S)          # [384, S]
    po[:S_TXT] = attf[:, :S_TXT].T @ w2e
    po[S_TXT:] = attf[:, S_TXT:].T @ w2i
    return po


def mock_kernel(**inputs):
    per_core = _prep(inputs)
    out = np.zeros((S, DM), F32)
    for d in per_core:
        out += _mock_core(d)
    enc_out = out[:S_TXT][None]
    img_out = out[S_TXT:][None]
    return img_out, enc_out


_CACHE = {}


def _build():
    if 'nc' in _CACHE:
        return _CACHE['nc']
    import concourse.bass as bass
    import concourse.mybir as mybir
    import concourse.tile as tile
    from concourse import bacc
    from contextlib import ExitStack

    f32 = mybir.dt.float32
    bf16 = mybir.dt.bfloat16
    nc = bacc.Bacc("TRN2", target_bir_lowering=False, debug=False,
                   num_devices=N_CORES)
    I = lambda n, sh, dt=bf16: nc.dram_tensor(n, sh, dt, kind="ExternalInput").ap()
    xt = I('xt', [DM, S])
    wq, wk = I('wq', [DM, DCH]), I('wk', [DM, DCH])
    waq, wak = I('waq', [DM, DCH]), I('wak', [DM, DCH])
    wv, wav = I('wv', [DM, DCH]), I('wav', [DM, DCH])
    aq, ak = I('aq', [DM, R]), I('ak', [DM, R])
    uq, uk = I('uq', [R, DCH]), I('uk', [R, DCH])
    cq, ck = I('cq', [R, 1], f32), I('ck', [R, 1], f32)
    bq_t, bk_t = I('bq_t', [128, NH], f32), I('bk_t', [128, NH], f32)
    baq_t, bak_t = I('baq_t', [128, NH], f32), I('bak_t', [128, NH], f32)
    bv_row, bav_row = I('bv_row', [1, DCH]), I('bav_row', [1, DCH])
    cosw_q, sinw_q = I('cosw_q', [128, S], f32), I('sinw_q', [128, S], f32)
    cosw_k, sinw_k = I('cosw_k', [128, S], f32), I('sinw_k', [128, S], f32)
    perm = I('perm', [128, 128])
    bok_t, dbk, dbv = I('bok_t', [DM, S_BOK]), I('dbk', [DM, R]), I('dbv', [DM, R])
    ukb, uvb = I('ukb', [R, DCH]), I('uvb', [R, DCH])
    wo_t, wao_t = I('wo_t', [DCH, DM]), I('wao_t', [DCH, DM])
    po = nc.dram_tensor('po', [S, DM], f32, kind="ExternalOutput").ap()

    ACT = mybir.ActivationFunctionType
    with nc.allow_low_precision(reason='bf16 kernel math'), \
         tile.TileContext(nc) as tc, ExitStack() as ctx:
        sing = ctx.enter_context(tc.tile_pool(name="sing", bufs=1))
        wpool = ctx.enter_context(tc.tile_pool(name="wpool", bufs=1))
        xs = ctx.enter_context(tc.tile_pool(name="xs", bufs=3))
        tmp = ctx.enter_context(tc.tile_pool(name="tmp", bufs=2))
        ropep = ctx.enter_context(tc.tile_pool(name="ropep", bufs=2))
        rows = ctx.enter_context(tc.tile_pool(name="rows", bufs=3))
        expp = ctx.enter_context(tc.tile_pool(name="expp", bufs=6))
        outp = ctx.enter_context(tc.tile_pool(name="outp", bufs=2))
        acc = ctx.enter_context(tc.tile_pool(name="acc", bufs=6, space="PSUM"))
        psm = ctx.enter_context(tc.tile_pool(name="psm", bufs=2, space="PSUM"))

        # constants
        ones_c_f = sing.tile([128, 1], f32, tag="ones_c_f")
        nc.vector.memset(ones_c_f, 1.0)
        ones_c_b = sing.tile([128, 1], bf16, tag="ones_c_b")
        nc.vector.memset(ones_c_b, 1.0)
        ones_r_f = sing.tile([1, 128], f32, tag="ones_r_f")
        nc.vector.memset(ones_r_f, 1.0)
        ones_r_b = sing.tile([1, 128], bf16, tag="ones_r_b")
        nc.vector.memset(ones_r_b, 1.0)
        eps_sb = sing.tile([1, 1], f32, tag="eps_sb")
        nc.vector.memset(eps_sb, EPS)
        perm_sb = sing.tile([128, 128], bf16, tag="perm_sb")
        nc.sync.dma_start(out=perm_sb, in_=perm)
        rope_ap = {'cq': cosw_q, 'sq': sinw_q, 'ck': cosw_k, 'sk': sinw_k}
        biases = {}
        for nm, apx in (('bq', bq_t), ('bk', bk_t), ('baq', baq_t), ('bak', bak_t)):
            t = sing.tile([128, NH], f32, tag=f"b_{nm}")
            nc.sync.dma_start(out=t, in_=apx)
            biases[nm] = t
        cq_sb = sing.tile([R, 1], f32, tag="cq_sb")
        nc.sync.dma_start(out=cq_sb, in_=cq)
        ck_sb = sing.tile([R, 1], f32, tag="ck_sb")
        nc.sync.dma_start(out=ck_sb, in_=ck)
        bvr = sing.tile([1, DCH], bf16, tag="bvr")
        nc.sync.dma_start(out=bvr, in_=bv_row)
        bavr = sing.tile([1, DCH], bf16, tag="bavr")
        nc.sync.dma_start(out=bavr, in_=bav_row)

        U = {}
        for nm, apx in (('uq', uq), ('uk', uk)):
            t = sing.tile([R, DCH], bf16, tag=f"u_{nm}")
            nc.sync.dma_start(out=t, in_=apx)
            U[nm] = t

        qtr = [sing.tile([128, S], bf16, tag=f"qtr{h}", name=f"qtr{h}") for h in range(NH)]
        ktr = [sing.tile([128, S], bf16, tag=f"ktr{h}", name=f"ktr{h}") for h in range(NH)]

        # ---- stage A: q/k projections + lora + rmsnorm + rope ----
        for c5 in range(NQC):
            is_enc = (c5 == 0)
            ssl = slice(c5 * 512, (c5 + 1) * 512)
            wq_ap = waq if is_enc else wq
            wk_ap = wak if is_enc else wk
            pq = [acc.tile([128, 512], f32, tag="acc", name=f"pq{c5}_{i}") for i in range(NH)]
            pk = [acc.tile([128, 512], f32, tag="acc", name=f"pk{c5}_{i}") for i in range(NH)]
            if not is_enc:
                pdq = psm.tile([R, 512], f32, tag="psm")
                pdk = psm.tile([R, 512], f32, tag="psm")
            for kt in range(NKT):
                ksl = slice(kt * 128, (kt + 1) * 128)
                xtile = xs.tile([128, 512], bf16, tag="xtile")
                nc.sync.dma_start(out=xtile, in_=xt[ksl, ssl])
                wq_t = xs.tile([128, DCH], bf16, tag="wq_t")
                nc.sync.dma_start(out=wq_t, in_=wq_ap[ksl, :])
                wk_t = xs.tile([128, DCH], bf16, tag="wk_t")
                nc.sync.dma_start(out=wk_t, in_=wk_ap[ksl, :])
                st = (kt == 0)
                for h in range(NH):
                    hsl = slice(h * 128, (h + 1) * 128)
                    nc.tensor.matmul(pq[h], wq_t[:, hsl], xtile,
                                     start=st, stop=(kt == NKT - 1 and is_enc))
                    nc.tensor.matmul(pk[h], wk_t[:, hsl], xtile,
                                     start=st, stop=(kt == NKT - 1 and is_enc))
                if not is_enc:
                    aq_t = xs.tile([128, R], bf16, tag="aq_t")
                    nc.sync.dma_start(out=aq_t, in_=aq[ksl, :])
                    ak_t = xs.tile([128, R], bf16, tag="ak_t")
                    nc.sync.dma_start(out=ak_t, in_=ak[ksl, :])
                    nc.tensor.matmul(pdq, aq_t, xtile, start=st, stop=(kt == NKT - 1))
                    nc.tensor.matmul(pdk, ak_t, xtile, start=st, stop=(kt == NKT - 1))
            if not is_enc:
                dq_sb = tmp.tile([R, 512], bf16, tag="d_sb")
                nc.scalar.activation(dq_sb, pdq, ACT.Identity, bias=cq_sb, scale=1.0)
                dk_sb = tmp.tile([R, 512], bf16, tag="d_sb")
                nc.scalar.activation(dk_sb, pdk, ACT.Identity, bias=ck_sb, scale=1.0)
                for h in range(NH):
                    hsl = slice(h * 128, (h + 1) * 128)
                    nc.tensor.matmul(pq[h], U['uq'][:, hsl], dq_sb, start=False, stop=True)
                    nc.tensor.matmul(pk[h], U['uk'][:, hsl], dk_sb, start=False, stop=True)
            cw = {}
            for nm in ('cq', 'sq', 'ck', 'sk'):
                t = ropep.tile([128, 512], f32, tag=f"rope_{nm}", name=f"rope_{nm}_{c5}")
                nc.sync.dma_start(out=t, in_=rope_ap[nm][:, ssl])
                cw[nm] = t
            for h in range(NH):
                for (ps, bnm, benm, cwn, swn, dst) in (
                        (pq[h], 'bq', 'baq', 'cq', 'sq', qtr[h]),
                        (pk[h], 'bk', 'bak', 'ck', 'sk', ktr[h])):
                    bt = biases[benm if is_enc else bnm]
                    raw = tmp.tile([128, 512], f32, tag="raw")
                    nc.scalar.activation(raw, ps, ACT.Identity, bias=bt[:, h:h + 1], scale=1.0)
                    sq_t = tmp.tile([128, 512], bf16, tag="sq_t")
                    nc.vector.tensor_mul(sq_t, raw, raw)
                    pss = psm.tile([1, 512], f32, tag="psm")
                    nc.tensor.matmul(pss, ones_c_b, sq_t, start=True, stop=True)
                    sq_row = rows.tile([1, 512], f32, tag="sq_row")
                    nc.scalar.activation(sq_row, pss, ACT.Sqrt, bias=eps_sb, scale=1.0 / 128.0)
                    r_row = rows.tile([1, 512], bf16, tag="r_row")
                    nc.vector.reciprocal(r_row, sq_row)
                    prb = psm.tile([128, 512], f32, tag="psm")
                    nc.tensor.matmul(prb, ones_r_b, r_row, start=True, stop=True)
                    rb_sbA = tmp.tile([128, 512], f32, tag="sq_t")
                    nc.scalar.activation(rb_sbA, prb, ACT.Copy)
                    qn = tmp.tile([128, 512], f32, tag="qn")
                    nc.vector.tensor_mul(qn, raw, rb_sbA)
                    qn_b = tmp.tile([128, 512], bf16, tag="qn_b")
                    nc.vector.tensor_copy(qn_b, qn)
                    prot = psm.tile([128, 512], f32, tag="psm")
                    nc.tensor.matmul(prot, perm_sb, qn_b, start=True, stop=True)
                    t1 = tmp.tile([128, 512], f32, tag="t1")
                    nc.vector.tensor_mul(t1, qn, cw[cwn])
                    t2 = tmp.tile([128, 512], f32, tag="t2")
                    nc.vector.tensor_mul(t2, prot, cw[swn])
                    nc.vector.tensor_add(dst[:, ssl], t1, t2)

        # ---- stage B: v ----
        vsb = sing.tile([128, NST, DCH], bf16, tag="vsb")
        for c5 in range(NQC):
            is_enc = (c5 == 0)
            wv_ap = wav if is_enc else wv
            pv = [acc.tile([128, DCH], f32, tag="acc", name=f"pv{c5}_{i}") for i in range(4)]
            for kt in range(NKT):
                ksl = slice(kt * 128, (kt + 1) * 128)
                xtile = xs.tile([128, 512], bf16, tag="xtile")
                nc.sync.dma_start(out=xtile, in_=xt[ksl, c5 * 512:(c5 + 1) * 512])
                wv_t = xs.tile([128, DCH], bf16, tag="wq_t")
                nc.sync.dma_start(out=wv_t, in_=wv_ap[ksl, :])
                for sb4 in range(4):
                    nc.tensor.matmul(pv[sb4], xtile[:, sb4 * 128:(sb4 + 1) * 128],
                                     wv_t, start=(kt == 0), stop=False)
            brow = bavr if is_enc else bvr
            for sb4 in range(4):
                nc.tensor.matmul(pv[sb4], ones_r_b, brow, start=False, stop=True)
                nc.scalar.activation(vsb[:, c5 * 4 + sb4, :], pv[sb4], ACT.Copy)

        # wv/wav loads (after stage A weights no longer needed — but pools are
        # static; wv/wav were preloaded above via W dict)

        # ---- bokeh small tensors ----
        bok_sb = sing.tile([128, NKT, S_BOK], bf16, tag="bok_sb")
        nc.sync.dma_start(out=bok_sb, in_=bok_t.rearrange("(t p) n -> p t n", p=128))
        dbk_sb = sing.tile([128, NKT, R], bf16, tag="dbk_sb")
        nc.sync.dma_start(out=dbk_sb, in_=dbk.rearrange("(t p) n -> p t n", p=128))
        dbv_sb = sing.tile([128, NKT, R], bf16, tag="dbv_sb")
        nc.sync.dma_start(out=dbv_sb, in_=dbv.rearrange("(t p) n -> p t n", p=128))
        ukb_sb = sing.tile([R, DCH], bf16, tag="ukb_sb")
        nc.sync.dma_start(out=ukb_sb, in_=ukb)
        uvb_sb = sing.tile([R, DCH], bf16, tag="uvb_sb")
        nc.sync.dma_start(out=uvb_sb, in_=uvb)
        pbk = psm.tile([R, S_BOK], f32, tag="psm")
        pbv = psm.tile([R, S_BOK], f32, tag="psm")
        for kt in range(NKT):
            nc.tensor.matmul(pbk, dbk_sb[:, kt, :], bok_sb[:, kt, :],
                             start=(kt == 0), stop=(kt == NKT - 1))
            nc.tensor.matmul(pbv, dbv_sb[:, kt, :], bok_sb[:, kt, :],
                             start=(kt == 0), stop=(kt == NKT - 1))
        dbk2 = rows.tile([R, S_BOK], bf16, tag="dbk2")
        nc.vector.tensor_copy(dbk2, pbk)
        dbv2 = rows.tile([R, S_BOK], bf16, tag="dbv2")
        nc.vector.tensor_copy(dbv2, pbv)
        kdT = sing.tile([128, NH * S_BOK], bf16, tag="kdT")
        for h in range(NH):
            pkd = psm.tile([128, S_BOK], f32, tag="psm")
            nc.tensor.matmul(pkd, ukb_sb[:, h * 128:(h + 1) * 128], dbk2,
                             start=True, stop=True)
            nc.vector.tensor_copy(kdT[:, h * S_BOK:(h + 1) * S_BOK], pkd)
        pvd = psm.tile([S_BOK, DCH], f32, tag="psm")
        nc.tensor.matmul(pvd, dbv2, uvb_sb, start=True, stop=True)
        vd_sb = sing.tile([S_BOK, DCH], bf16, tag="vd_sb")
        nc.vector.tensor_copy(vd_sb, pvd)

        # out-proj weights (loaded during attention)
        W2 = {}
        for nm, apx in (('wo', wo_t), ('wao', wao_t)):
            t = wpool.tile([128, NH, DM], bf16, tag=f"w2_{nm}")
            nc.sync.dma_start(out=t, in_=apx.rearrange("(h p) n -> p h n", p=128))
            W2[nm] = t

        # ---- attention ----
        att_tot = [sing.tile([128, S], bf16, tag=f"att{h}", name=f"att{h}") for h in range(NH)]
        for h in range(NH):
            hsl = slice(h * 128, (h + 1) * 128)
            for qc in range(NQC):
                qsl = slice(qc * 512, (qc + 1) * 512)
                psum_s = psm.tile([1, 512], f32, tag="psm")
                p_att = acc.tile([128, 512], f32, tag="acc")
                for kt in range(NST):
                    ps = acc.tile([128, 512], f32, tag="acc")
                    nc.tensor.matmul(ps, ktr[h][:, kt * 128:(kt + 1) * 128],
                                     qtr[h][:, qsl], start=True, stop=True)
                    ex = expp.tile([128, 512], bf16, tag="ex")
                    nc.scalar.activation(ex, ps, ACT.Exp, scale=SCALE)
                    nc.tensor.matmul(psum_s, ones_c_b, ex,
                                     start=(kt == 0), stop=(kt == NST - 1))
                    nc.tensor.matmul(p_att, vsb[:, kt, hsl], ex,
                                     start=(kt == 0), stop=(kt == NST - 1))
                psb = psm.tile([S_BOK, 512], f32, tag="psm")
                nc.tensor.matmul(psb, kdT[:, h * S_BOK:(h + 1) * S_BOK],
                                 qtr[h][:, qsl], start=True, stop=True)
                exb = expp.tile([S_BOK, 512], bf16, tag="exb")
                nc.scalar.activation(exb, psb, ACT.Exp, scale=SCALE)
                psum_sb = psm.tile([1, 512], f32, tag="psm")
                nc.tensor.matmul(psum_sb, ones_c_b[0:S_BOK, :], exb, start=True, stop=True)
                p_cam = acc.tile([128, 512], f32, tag="acc")
                nc.tensor.matmul(p_cam, vd_sb[:, hsl], exb, start=True, stop=True)
                r_m = rows.tile([1, 512], bf16, tag="r_m")
                nc.vector.reciprocal(r_m, psum_s)
                r_b = rows.tile([1, 512], bf16, tag="r_b")
                nc.vector.reciprocal(r_b, psum_sb)
                prm = acc.tile([128, 512], f32, tag="acc")
                nc.tensor.matmul(prm, ones_r_b, r_m, start=True, stop=True)
                prb2 = acc.tile([128, 512], f32, tag="acc")
                nc.tensor.matmul(prb2, ones_r_b, r_b, start=True, stop=True)
                rm_sb = tmp.tile([128, 512], f32, tag="raw")
                nc.scalar.activation(rm_sb, prm, ACT.Copy)
                rb_sb = tmp.tile([128, 512], f32, tag="sq_t")
                nc.scalar.activation(rb_sb, prb2, ACT.Copy)
                t1 = tmp.tile([128, 512], f32, tag="t1")
                nc.vector.tensor_mul(t1, p_att, rm_sb)
                t2 = tmp.tile([128, 512], f32, tag="t2")
                nc.vector.tensor_mul(t2, p_cam, rb_sb)
                nc.vector.tensor_add(att_tot[h][:, qsl], t1, t2)

        # ---- out-proj ----
        for qt in range(NST):
            is_enc = qt < 4
            w2 = W2['wao'] if is_enc else W2['wo']
            for nch in range(6):
                nsl = slice(nch * 512, (nch + 1) * 512)
                pso = acc.tile([128, 512], f32, tag="acc")
                for h in range(NH):
                    nc.tensor.matmul(pso, att_tot[h][:, qt * 128:(qt + 1) * 128],
                                     w2[:, h, nsl], start=(h == 0), stop=(h == NH - 1))
                osb = outp.tile([128, 512], f32, tag="osb")
                nc.vector.tensor_copy(osb, pso)
                nc.sync.dma_start(out=po[qt * 128:(qt + 1) * 128, nsl], in_=osb)

    nc.finalize()
    _CACHE['nc'] = nc
    return nc


def kernel(**inputs):
    from concourse import bass_utils
    per_core = _prep(inputs)
    nc = _build()
    try:
        res = bass_utils.run_bass_kernel_spmd(nc, per_core, core_ids=list(range(N_CORES)))
    except Exception:
        res = bass_utils.run_bass_kernel_spmd(nc, per_core, core_ids=list(range(N_CORES)))
    out = np.zeros((S, DM), F32)
    for i in range(N_CORES):
        out += res.results[i]['po']
    out[:S_TXT] += np.asarray(inputs['bao'], F32)
    out[S_TXT:] += np.asarray(inputs['bo'], F32)
    return out[S_TXT:][None].copy(), out[:S_TXT][None].copy()
